# revision 15
# baseline (speedup 1.0000x reference)
"""Trainium2 Bass kernel for nn_DistanceLoss (patch neighbor-distance loss).

Reference semantics (k=16, H=W=2048, LOSS_WEIGHT=1):
  split each image into non-overlapping 16x16 patches; for interior pixels
  (local i,j in 1..14) and the 8-neighbor offset list [E,NW,NE,N,E,SW,SE,S]
  (E twice, W missing), accumulate || |sr_c-sr_n| - |hr_c-hr_n| || and take
  the global mean over L*14*14*8 terms.

Core trick: the per-term value t = ||u|-|v|| (u = sr_c-sr_n, v = hr_c-hr_n)
is three chained ABSOLUTE_DIFF ALU ops. The host stages sr/hr INTERLEAVED
(x[2f]=sr, x[2f+1]=hr) so that in the DVE's fp16 2x packed mode one
instruction sees all four operands per cycle (SRC_0=sr_x, SRC_0_HI=hr_x,
SRC_1=sr_{x+o}, SRC_1_HI=hr_{x+o}) and emits t duplicated to both write
lanes. This toolchain's walrus predates the CUSTOM_DVE_ANT opcodes, so the
custom 3-stage uop program is installed by HIJACKING the stock
TENSOR_TENSOR_ARITH_OP (0x41) row of the per-NEFF DVE table (the stock
sequencer handler already enables the two-source perf mode, which makes the
engine pick the 2x_1P uop slot for fp16 contiguous operands). Every
nc.vector.tensor_tensor in this kernel therefore computes the fused
pair-absdiff, one t per cycle per lane -- there is no S/D construction, no
shifted-copy DMA, no separate abs or min pass, and the Scalar engine is
freed up to issue half the input DMAs.

Opposite offsets +o/-o share one t array (sum over shifted windows), so the
pairs {N,S}, {NW,SE}, {NE,SW} cost one pass each and E (listed twice) has
weight 2. The interior-window sums run on PE as ones/twos-weighted
[128,1]^T @ t-row matmuls accumulating into PSUM [1,224]; rhs APs read the
duplicated t tiles with an inner stride of 2 so each t counts once. The
last pair (E) writes two tiles so PE can start its tail matmuls early.

Sharding: 256 image columns per core (16 patch-cols x 128 patch-rows),
free index f = i*256 + c; every neighbor offset is the constant
interleaved shift 2*(di*256+dj), always 4-byte aligned.
"""

import numpy as np

H = W = 2048
K = 16
NCORES = 8
WC = W // NCORES          # 256 columns per core
FREE = K * WC             # 4096 f-positions per partition
WIN = 15 * WC             # 3840: compute window covers i = 0..14
XPAD = 8208               # x tile width (2*FREE + junk tail for o=257 reads)
N_TERMS = (H // K) * (W // K) * (K - 2) * (K - 2) * 8
SPLIT_A = 1536            # A/B pass boundary (multiple of 256: row-aligned)

TT_ARITH_OPCODE = 0x41    # stock TENSOR_TENSOR_ARITH_OP row we repoint
PAIRMIN_NAME = "PAIRMIN_TT_ANT"


def _split_multiwaits(nc):
    """The walrus build here accepts at most one sync wait (and one update)
    per instruction: hoist extra waits onto same-engine NoOps inserted
    before the instruction, and extra updates onto NoOps after it."""
    from concourse import mybir

    k = 0
    for f in nc.m.functions:
        for bb in f.blocks:
            out, changed = [], False
            for i in bb.instructions:
                si = i.sync_info
                waits = list(si.on_wait) if si else []
                ups = list(si.on_update) if si else []
                trimmed = False
                if len(waits) > 1:
                    for w in waits[:-1]:
                        n = mybir.InstNoOp(name=f"{i.name}-sw{k}", ins=[],
                                           outs=[])
                        k += 1
                        n.engine = i.engine
                        n.sync_info = mybir.SyncInfo(on_wait=[w], on_update=[])
                        out.append(n)
                    waits, changed, trimmed = waits[-1:], True, True
                out.append(i)
                if len(ups) > 1:
                    i.sync_info = mybir.SyncInfo(on_wait=waits,
                                                 on_update=ups[:1])
                    for u in ups[1:]:
                        n = mybir.InstNoOp(name=f"{i.name}-su{k}", ins=[],
                                           outs=[])
                        k += 1
                        n.engine = i.engine
                        n.sync_info = mybir.SyncInfo(on_wait=[], on_update=[u])
                        out.append(n)
                    changed = True
                elif trimmed:
                    i.sync_info = mybir.SyncInfo(on_wait=waits, on_update=ups)
            if changed:
                bb.instructions = out
    return k


def _pairmin_ref(in0, in1, s0, s1, imm2):
    """numpy semantics of the hijacked op (sim/IR reference)."""
    a, b = in0[..., 0::2].astype(np.float32), in0[..., 1::2].astype(np.float32)
    c, d = in1[..., 0::2].astype(np.float32), in1[..., 1::2].astype(np.float32)
    t = np.abs(np.abs(a - c) - np.abs(b - d))
    return np.repeat(t, 2, axis=-1)


def _register_pairmin():
    """Install PAIRMIN into dve_ops.OPS with a hand-built 2x_1P uop program
    keyed to the stock TENSOR_TENSOR opcode row."""
    from concourse.dve_spec import Spec, Src0, Src1, Bin, lower
    from concourse.dve_uop import (
        UopConfig, DveOpSpec, InpSel, OutPath, OutSel,
        AluInp, AluOp, DelayInp, Trigger, ENABLE,
    )
    from concourse.dve_ops import DveOp, OPS, CUSTOM_DVE_SPECS, _COMPILE_CACHE

    if any(op.name == PAIRMIN_NAME for op in OPS):
        return

    u = UopConfig()
    u.inp[0], u.inp_enable[0] = InpSel.SRC_0, ENABLE       # sr_x
    u.inp[1], u.inp_enable[1] = InpSel.SRC_1, ENABLE       # sr_{x+o}
    u.inp[2], u.inp_enable[2] = InpSel.SRC_0_HI, ENABLE    # hr_x
    u.inp[3], u.inp_enable[3] = InpSel.SRC_1_HI, ENABLE    # hr_{x+o}
    dp = u.datapath_config
    dp[0].enable_alu(AluOp.ABSOLUTE_DIFF, AluInp.PREV_ALU_OUT,
                     AluInp.PREV_DELAY_0)
    dp[0].pass_through_delay(1, 2)
    dp[1].enable_alu(AluOp.ABSOLUTE_DIFF, AluInp.PREV_DELAY_1,
                     AluInp.PREV_DELAY_2)
    dp[1].enable_delay_from_src(DelayInp.PREV_ALU_OUT, 0)
    dp[2].enable_alu(AluOp.ABSOLUTE_DIFF, AluInp.PREV_ALU_OUT,
                     AluInp.PREV_DELAY_0)
    for k in range(3, 8):
        dp[k].pass_through_alu()
    u.out[OutPath.WR0_LO], u.out_enable[OutPath.WR0_LO] = OutSel.ALU_OUT, ENABLE
    u.out[OutPath.WR0_HI], u.out_enable[OutPath.WR0_HI] = OutSel.ALU_OUT, ENABLE
    u.require_inp0 = 1
    u.require_inp1 = 1
    u.trigger = (Trigger.SRC_TENSOR_DONE, Trigger.NONE, Trigger.NONE)

    op = DveOp(PAIRMIN_NAME,
               Spec(body=Bin(AluOp.ABSOLUTE_DIFF, Src0, Src1),
                    reference=_pairmin_ref),
               subdim=False, uops_sha={})
    OPS.append(op)
    CUSTOM_DVE_SPECS[PAIRMIN_NAME] = op.spec
    reg = lower(op.spec, ver="v3")
    assert len(reg) == 1
    _COMPILE_CACHE[(PAIRMIN_NAME, "v3")] = DveOpSpec(
        name=PAIRMIN_NAME, opcode=TT_ARITH_OPCODE, uops=reg,
        uops_2x=[u], perf_max=1, rd1_en=True)


def _build_bass():
    from concourse import bass, mybir, tile

    _register_pairmin()

    nc = bass.Bass()
    # block-major staging: A = cols 0:4096, B = cols 4096:8192, each split
    # 96/32 by partitions (sync HWDGE starts ~1.7us earlier than scalar, so
    # it gets the bigger share). 8KB sequential-HBM descriptors throughout.
    x_in = nc.declare_dram_parameter("x_in", [4 * 64, 4096],
                                     mybir.dt.float16, isOutput=False)
    out_sum = nc.declare_dram_parameter("out_sum", [1, 8],
                                        mybir.dt.float32, isOutput=True)
    nc.m.ant_custom_dve_ops = sorted({*nc.m.ant_custom_dve_ops, PAIRMIN_NAME})

    fp16 = mybir.dt.float16
    f32 = mybir.dt.float32
    Alu = mybir.AluOpType

    with tile.TileContext(nc) as tc:
        with tc.tile_pool(name="io", bufs=1) as io_pool, \
             tc.tile_pool(name="tpool", bufs=4) as t_pool, \
             tc.tile_pool(name="psum", bufs=1, space="PSUM") as psum_pool:
            x = io_pool.tile([128, XPAD], fp16, tag="x")
            w1 = io_pool.tile([128, 1], fp16, tag="w1")
            w2 = io_pool.tile([128, 1], fp16, tag="w2")
            acc = psum_pool.tile([1, 256], f32, tag="acc")
            colsb = io_pool.tile([1, 8], f32, tag="colsb")

            nc.vector.memset(w1[:, :], 1.0)
            nc.vector.memset(w2[:, :], 2.0)

            # 4 block loads (block-major DRAM source): A-phase passes need
            # x[:4096] (first two blocks); B-phase needs the rest.
            for col in (0, 4096):
                r = col // 32  # 0 or 128: row base in the staged layout
                nc.sync.dma_start(out=x[0:96, col:col + 4096],
                                  in_=x_in[r:r + 96, :])
                nc.scalar.dma_start(out=x[96:128, col:col + 4096],
                                    in_=x_in[r + 96:r + 128, :])

            def rows_w():
                return [(1.0 if (i == 0 or i == 14) else 2.0)
                        for i in range(15)]

            # (offset, window lo, PE plan) in issue order; plan entries:
            # ("mid", j_lo, j_hi, row_weights, row_lo, row_hi) weighted row
            # matmuls, ("emid", ...) the x2-weighted E rows,
            # ("strip", j, row_lo, row_hi) single-column edge matmuls.
            PAIRS = [
                (256, 0, [("mid", 1, 15, rows_w(), 0, 15)]),
                (255, 0, [("mid", 2, 15, rows_w(), 0, 15),
                          ("strip", 1, 1, 15),
                          ("strip", 15, 0, 14)]),
                (257, 0, [("mid", 1, 14, rows_w(), 0, 15),
                          ("strip", 14, 1, 15),
                          ("strip", 0, 0, 14)]),
                (1, WC, [("emid", 1, 15, None, 1, 15)]),
            ]

            first_mm = [True]

            def mm(rhs, wts, stop=False):
                width = int(np.prod(rhs.shape[1:]))
                nc.tensor.matmul(acc[:, 0:width], wts[:, :], rhs,
                                 start=first_mm[0], stop=stop)
                first_mm[0] = False

            # fused pair-absdiff pass over f-window [flo, fhi): one hijacked
            # tensor_tensor on the interleaved tile. dst holds (t,t) pairs.
            def pair_pass(t_tile, tbase, flo, fhi, o):
                nc.vector.tensor_tensor(
                    t_tile[:, 2 * (flo - tbase):2 * (fhi - tbase)],
                    x[:, 2 * flo:2 * fhi],
                    x[:, 2 * (flo + o):2 * (fhi + o)], Alu.add)

            # A phase (needs x[:4096] = chunks 0-3)
            tiles = []
            for o, oplo, plan in PAIRS[:3]:
                t = t_pool.tile([128, 2 * WIN], fp16, tag="t")
                tiles.append(t)
                pair_pass(t, 0, oplo, SPLIT_A, o)
            t_a = t_pool.tile([128, 2 * 2048], fp16, tag="ta")
            t_b1 = t_pool.tile([128, 2 * 1024], fp16, tag="tb1")
            t_b2 = t_pool.tile([128, 2 * 512], fp16, tag="tb2")
            t_b3 = t_pool.tile([128, 2 * 256], fp16, tag="tb3")
            # E rows 1..7 except f=2047 (i=7,j=15, never read by the plan):
            # keeps the A pass inside x[:4096].
            pair_pass(t_a, 0, WC, 2047, 1)

            # B phase (needs the full input). E's tail rows go to separate
            # small tiles so PE can chase the last DVE ops closely.
            for (o, oplo, plan), t in zip(PAIRS[:3], tiles):
                pair_pass(t, 0, SPLIT_A, WIN, o)
            pair_pass(t_b1, 2048, 2048, 3072, 1)   # rows 8-11
            pair_pass(t_b2, 3072, 3072, 3584, 1)   # rows 12-13
            pair_pass(t_b3, 3584, 3584, WIN, 1)    # row 14

            # PE reductions. Views: i rows x 16 patches x 16 cols x 2 dups.
            def views(tile_, irows):
                v5 = tile_.rearrange("p (i q j d) -> p i q j d",
                                     q=16, j=16, d=2)
                v4 = tile_.rearrange("p (i q jd) -> p i q jd", q=16, jd=32)
                return v5, v4

            for pi, ((o, oplo, plan), t) in enumerate(
                    zip(PAIRS[:3], tiles)):
                v5, v4 = views(t, 15)
                for e in plan:
                    if e[0] == "mid":
                        _, a, b, wts, rlo, rhi = e
                        for i in range(rlo, rhi):
                            w = w1 if wts[i] == 1.0 else w2
                            mm(v5[:, i, :, a:b, 0:1], w)
                    else:  # ("strip", j, row_lo, row_hi)
                        _, j, rlo, rhi = e
                        mm(v4[:, rlo:rhi, :, 2 * j:2 * j + 1], w1)

            # E: rows 1..7 from t_a (ready after the A phase), then the tail
            # tiles; only row 14's matmul trails the final DVE op.
            va5, _ = views(t_a, 8)
            vb5, _ = views(t_b1, 4)
            vc5, _ = views(t_b2, 2)
            vd5, _ = views(t_b3, 1)
            for i in range(1, 15):
                if i < 8:
                    v = va5[:, i]
                elif i < 12:
                    v = vb5[:, i - 8]
                elif i < 14:
                    v = vc5[:, i - 12]
                else:
                    v = vd5[:, 0]
                mm(v[:, :, 1:15, 0:1], w2, stop=(i == 14))

            # drain PSUM to a scalar
            nc.vector.tensor_reduce(colsb[:, 0:1], acc[:, 0:224],
                                    mybir.AxisListType.X, Alu.add)
            nc.sync.dma_start(out=out_sum[:, :], in_=colsb[:, :])
    _split_multiwaits(nc)
    return nc


_NC_CACHE = None
LAST_RESULTS = None  # BassKernelResults of the most recent run (for test.py)


def kernel(sr_tensor: np.ndarray, hr_tensor: np.ndarray) -> np.ndarray:
    from concourse.bass_utils import run_bass_kernel_spmd

    global _NC_CACHE, LAST_RESULTS
    if _NC_CACHE is None:
        _NC_CACHE = _build_bass()
    nc = _NC_CACHE

    sr = np.asarray(sr_tensor, dtype=np.float32).reshape(H, W)
    hr = np.asarray(hr_tensor, dtype=np.float32).reshape(H, W)

    in_maps = []
    for c in range(NCORES):
        c0 = c * WC
        # [2048, 256] -> [128 patch-rows, 16 rows, 256 cols] -> interleave
        s16 = sr[:, c0:c0 + WC].reshape(128, FREE).astype(np.float16)
        h16 = hr[:, c0:c0 + WC].reshape(128, FREE).astype(np.float16)
        xi = np.empty((128, FREE, 2), dtype=np.float16)
        xi[:, :, 0] = s16
        xi[:, :, 1] = h16
        xi = xi.reshape(128, 2 * FREE)
        # block-major, 96/32 partition split per column half
        xs = np.concatenate([xi[0:96, 0:4096], xi[96:128, 0:4096],
                             xi[0:96, 4096:8192], xi[96:128, 4096:8192]],
                            axis=0)
        in_maps.append({"x_in": np.ascontiguousarray(xs)})

    res = run_bass_kernel_spmd(nc, in_maps, list(range(NCORES)))
    LAST_RESULTS = res

    total = 0.0
    for r in res.results:
        total += float(np.asarray(r["out_sum"], dtype=np.float64)[0, 0])
    return np.float32(total / N_TERMS)


# revision 21
# speedup vs baseline: 1.0910x; 1.0910x over previous
"""Trainium2 Bass kernel for nn_DistanceLoss (patch neighbor-distance loss).

Reference semantics (k=16, H=W=2048, LOSS_WEIGHT=1):
  split each image into non-overlapping 16x16 patches; for interior pixels
  (local i,j in 1..14) and the 8-neighbor offset list [E,NW,NE,N,E,SW,SE,S]
  (E twice, W missing), accumulate || |sr_c-sr_n| - |hr_c-hr_n| || and take
  the global mean over L*14*14*8 terms.

Core trick: the per-term value t = ||u|-|v|| (u = sr_c-sr_n, v = hr_c-hr_n)
is three chained ABSOLUTE_DIFF ALU ops. The host stages sr/hr INTERLEAVED
(x[2f]=sr, x[2f+1]=hr) so that in the DVE's fp16 2x packed mode one
instruction sees all four operands per cycle (SRC_0=sr_x, SRC_0_HI=hr_x,
SRC_1=sr_{x+o}, SRC_1_HI=hr_{x+o}) and emits t duplicated to both write
lanes. This toolchain's walrus predates the CUSTOM_DVE_ANT opcodes, so the
custom 3-stage uop program is installed by HIJACKING the stock
TENSOR_TENSOR_ARITH_OP (0x41) row of the per-NEFF DVE table (the stock
sequencer handler already enables the two-source perf mode, which makes the
engine pick the 2x_1P uop slot for fp16 contiguous operands). Every
nc.vector.tensor_tensor in this kernel therefore computes the fused
pair-absdiff, one t per cycle per lane -- there is no S/D construction, no
shifted-copy DMA, no separate abs or min pass, and the Scalar engine is
freed up to issue half the input DMAs.

Opposite offsets +o/-o share one t array (sum over shifted windows), so the
pairs {N,S}, {NW,SE}, {NE,SW} cost one pass each and E (listed twice) has
weight 2. The interior-window sums run on PE as ones/twos-weighted
[128,1]^T @ t-row matmuls accumulating into PSUM [1,224]; rhs APs read the
duplicated t tiles with an inner stride of 2 so each t counts once. The
last pair (E) writes two tiles so PE can start its tail matmuls early.

Sharding: 256 image columns per core (16 patch-cols x 128 patch-rows),
free index f = i*256 + c; every neighbor offset is the constant
interleaved shift 2*(di*256+dj), always 4-byte aligned.
"""

import numpy as np

H = W = 2048
K = 16
NCORES = 8
WC = W // NCORES          # 256 columns per core
FREE = K * WC             # 4096 f-positions per partition
WIN = 15 * WC             # 3840: compute window covers i = 0..14
XPAD = 8208               # x tile width (2*FREE + junk tail for o=257 reads)
N_TERMS = (H // K) * (W // K) * (K - 2) * (K - 2) * 8
SPLIT_A = 1536            # A/B pass boundary (multiple of 256: row-aligned)

TT_ARITH_OPCODE = 0x41    # stock TENSOR_TENSOR_ARITH_OP row we repoint
PAIRMIN_NAME = "PAIRMIN_TT_ANT"


def _split_multiwaits(nc):
    """The walrus build here accepts at most one sync wait (and one update)
    per instruction: hoist extra waits onto same-engine NoOps inserted
    before the instruction, and extra updates onto NoOps after it."""
    from concourse import mybir

    k = 0
    for f in nc.m.functions:
        for bb in f.blocks:
            out, changed = [], False
            for i in bb.instructions:
                si = i.sync_info
                waits = list(si.on_wait) if si else []
                ups = list(si.on_update) if si else []
                trimmed = False
                if len(waits) > 1:
                    for w in waits[:-1]:
                        n = mybir.InstNoOp(name=f"{i.name}-sw{k}", ins=[],
                                           outs=[])
                        k += 1
                        n.engine = i.engine
                        n.sync_info = mybir.SyncInfo(on_wait=[w], on_update=[])
                        out.append(n)
                    waits, changed, trimmed = waits[-1:], True, True
                out.append(i)
                if len(ups) > 1:
                    i.sync_info = mybir.SyncInfo(on_wait=waits,
                                                 on_update=ups[:1])
                    for u in ups[1:]:
                        n = mybir.InstNoOp(name=f"{i.name}-su{k}", ins=[],
                                           outs=[])
                        k += 1
                        n.engine = i.engine
                        n.sync_info = mybir.SyncInfo(on_wait=[], on_update=[u])
                        out.append(n)
                    changed = True
                elif trimmed:
                    i.sync_info = mybir.SyncInfo(on_wait=waits, on_update=ups)
            if changed:
                bb.instructions = out
    return k


def _pairmin_ref(in0, in1, s0, s1, imm2):
    """numpy semantics of the hijacked op (sim/IR reference)."""
    a, b = in0[..., 0::2].astype(np.float32), in0[..., 1::2].astype(np.float32)
    c, d = in1[..., 0::2].astype(np.float32), in1[..., 1::2].astype(np.float32)
    t = np.abs(np.abs(a - c) - np.abs(b - d))
    return np.repeat(t, 2, axis=-1)


def _register_pairmin():
    """Install PAIRMIN into dve_ops.OPS with a hand-built 2x_1P uop program
    keyed to the stock TENSOR_TENSOR opcode row."""
    from concourse.dve_spec import Spec, Src0, Src1, Bin, lower
    from concourse.dve_uop import (
        UopConfig, DveOpSpec, InpSel, OutPath, OutSel,
        AluInp, AluOp, DelayInp, Trigger, ENABLE,
    )
    from concourse.dve_ops import DveOp, OPS, CUSTOM_DVE_SPECS, _COMPILE_CACHE

    if any(op.name == PAIRMIN_NAME for op in OPS):
        return

    u = UopConfig()
    u.inp[0], u.inp_enable[0] = InpSel.SRC_0, ENABLE       # sr_x
    u.inp[1], u.inp_enable[1] = InpSel.SRC_1, ENABLE       # sr_{x+o}
    u.inp[2], u.inp_enable[2] = InpSel.SRC_0_HI, ENABLE    # hr_x
    u.inp[3], u.inp_enable[3] = InpSel.SRC_1_HI, ENABLE    # hr_{x+o}
    dp = u.datapath_config
    dp[0].enable_alu(AluOp.ABSOLUTE_DIFF, AluInp.PREV_ALU_OUT,
                     AluInp.PREV_DELAY_0)
    dp[0].pass_through_delay(1, 2)
    dp[1].enable_alu(AluOp.ABSOLUTE_DIFF, AluInp.PREV_DELAY_1,
                     AluInp.PREV_DELAY_2)
    dp[1].enable_delay_from_src(DelayInp.PREV_ALU_OUT, 0)
    dp[2].enable_alu(AluOp.ABSOLUTE_DIFF, AluInp.PREV_ALU_OUT,
                     AluInp.PREV_DELAY_0)
    for k in range(3, 8):
        dp[k].pass_through_alu()
    u.out[OutPath.WR0_LO], u.out_enable[OutPath.WR0_LO] = OutSel.ALU_OUT, ENABLE
    u.out[OutPath.WR0_HI], u.out_enable[OutPath.WR0_HI] = OutSel.ALU_OUT, ENABLE
    u.require_inp0 = 1
    u.require_inp1 = 1
    u.trigger = (Trigger.SRC_TENSOR_DONE, Trigger.NONE, Trigger.NONE)

    op = DveOp(PAIRMIN_NAME,
               Spec(body=Bin(AluOp.ABSOLUTE_DIFF, Src0, Src1),
                    reference=_pairmin_ref),
               subdim=False, uops_sha={})
    OPS.append(op)
    CUSTOM_DVE_SPECS[PAIRMIN_NAME] = op.spec
    reg = lower(op.spec, ver="v3")
    assert len(reg) == 1
    _COMPILE_CACHE[(PAIRMIN_NAME, "v3")] = DveOpSpec(
        name=PAIRMIN_NAME, opcode=TT_ARITH_OPCODE, uops=reg,
        uops_2x=[u], perf_max=1, rd1_en=True)


def _build_bass():
    from concourse import bass, mybir, tile

    _register_pairmin()

    nc = bass.Bass()
    # block-major staging: A = cols 0:4096 (sync queue, which starts ~1us
    # after issue), B = cols 4096:8192 (scalar queue, ~3us doorbell latency
    # but B isn't needed until ~7us after A). 8KB sequential-HBM
    # descriptors throughout.
    x_in = nc.declare_dram_parameter("x_in", [2 * 128, 4096],
                                     mybir.dt.float16, isOutput=False)
    out_sum = nc.declare_dram_parameter("out_sum", [1, 8],
                                        mybir.dt.float32, isOutput=True)
    nc.m.ant_custom_dve_ops = sorted({*nc.m.ant_custom_dve_ops, PAIRMIN_NAME})

    fp16 = mybir.dt.float16
    f32 = mybir.dt.float32
    Alu = mybir.AluOpType

    with tile.TileContext(nc) as tc:
        with tc.tile_pool(name="io", bufs=1) as io_pool, \
             tc.tile_pool(name="tpool", bufs=4) as t_pool, \
             tc.tile_pool(name="psum", bufs=1, space="PSUM") as psum_pool:
            x = io_pool.tile([128, XPAD], fp16, tag="x")
            w1 = io_pool.tile([128, 1], fp16, tag="w1")
            w2 = io_pool.tile([128, 1], fp16, tag="w2")
            acc = psum_pool.tile([1, 256], f32, tag="acc")
            colsb = io_pool.tile([1, 8], f32, tag="colsb")

            nc.vector.memset(w1[:, :], 1.0)
            nc.vector.memset(w2[:, :], 2.0)

            # A block on sync, B block on scalar (both issue immediately;
            # transfers run concurrently on separate DGE queues).
            nc.sync.dma_start(out=x[:, 0:4096], in_=x_in[0:128, :])
            nc.scalar.dma_start(out=x[:, 4096:8192], in_=x_in[128:256, :])

            def rows_w():
                return [(1.0 if (i == 0 or i == 14) else 2.0)
                        for i in range(15)]

            # (offset, window lo, PE plan) in issue order; plan entries:
            # ("mid", j_lo, j_hi, row_weights, row_lo, row_hi) weighted row
            # matmuls, ("emid", ...) the x2-weighted E rows,
            # ("strip", j, row_lo, row_hi) single-column edge matmuls.
            PAIRS = [
                (256, 0, [("mid", 1, 15, rows_w(), 0, 15)]),
                (255, 0, [("mid", 2, 15, rows_w(), 0, 15),
                          ("strip", 1, 1, 15),
                          ("strip", 15, 0, 14)]),
                (257, 0, [("mid", 1, 14, rows_w(), 0, 15),
                          ("strip", 14, 1, 15),
                          ("strip", 0, 0, 14)]),
                (1, WC, [("emid", 1, 15, None, 1, 15)]),
            ]

            first_mm = [True]

            def mm(rhs, wts, stop=False):
                width = int(np.prod(rhs.shape[1:]))
                nc.tensor.matmul(acc[:, 0:width], wts[:, :], rhs,
                                 start=first_mm[0], stop=stop)
                first_mm[0] = False

            # fused pair-absdiff pass over f-window [flo, fhi): one hijacked
            # tensor_tensor on the interleaved tile. dst holds (t,t) pairs.
            def pair_pass(t_tile, tbase, flo, fhi, o):
                nc.vector.tensor_tensor(
                    t_tile[:, 2 * (flo - tbase):2 * (fhi - tbase)],
                    x[:, 2 * flo:2 * fhi],
                    x[:, 2 * (flo + o):2 * (fhi + o)], Alu.add)

            # A phase (needs x[:4096])
            tiles = []
            for o, oplo, plan in PAIRS[:3]:
                t = t_pool.tile([128, 2 * WIN], fp16, tag="t")
                tiles.append(t)
                pair_pass(t, 0, oplo, SPLIT_A, o)
            t_a = t_pool.tile([128, 2 * 2048], fp16, tag="ta")
            t_b = t_pool.tile([128, 2 * (WIN - 2048)], fp16, tag="tb")
            # E rows 1..7 except f=2047 (i=7,j=15, never read by the plan):
            # keeps the A pass inside x[:4096].
            pair_pass(t_a, 0, WC, 2047, 1)

            # B phase (needs the full input), ordered so heavy-PE passes
            # finish first and the final pass (p256 rows 11-14) leaves only
            # four matmuls trailing.
            t256, t255, t257 = tiles
            pair_pass(t255, 0, SPLIT_A, WIN, 255)
            pair_pass(t257, 0, SPLIT_A, WIN, 257)
            pair_pass(t_b, 2048, 2048, WIN, 1)        # E rows 8-14
            pair_pass(t256, 0, SPLIT_A, 2816, 256)    # rows 6-10
            pair_pass(t256, 0, 2816, WIN, 256)        # rows 11-14

            # PE reductions. Views: i rows x 16 patches x 16 cols x 2 dups.
            def views(tile_, irows):
                v5 = tile_.rearrange("p (i q j d) -> p i q j d",
                                     q=16, j=16, d=2)
                v4 = tile_.rearrange("p (i q jd) -> p i q jd", q=16, jd=32)
                return v5, v4

            def emit_pair(pi, rlo, rhi, strips, stop=False):
                o, oplo, plan = PAIRS[pi]
                v5, v4 = views(tiles[pi], 15)
                for e in plan:
                    if e[0] == "mid":
                        _, a, b, wts, mlo, mhi = e
                        lo, hi = max(rlo, mlo), min(rhi, mhi)
                        for i in range(lo, hi):
                            w = w1 if wts[i] == 1.0 else w2
                            mm(v5[:, i, :, a:b, 0:1], w,
                               stop=stop and i == hi - 1)
                    elif strips:  # ("strip", j, row_lo, row_hi)
                        _, j, slo, shi = e
                        mm(v4[:, slo:shi, :, 2 * j:2 * j + 1], w1)

            va5, _ = views(t_a, 8)
            vb5, _ = views(t_b, 7)

            def emit_e(rlo, rhi):
                for i in range(rlo, rhi):
                    v = va5[:, i] if i < 8 else vb5[:, i - 8]
                    mm(v[:, :, 1:15, 0:1], w2)

            # availability order: A-phase work, then B passes as above
            emit_pair(0, 0, 6, strips=False)
            emit_pair(1, 0, 6, strips=False)
            emit_pair(2, 0, 6, strips=False)
            emit_e(1, 8)
            emit_pair(1, 6, 15, strips=True)
            emit_pair(2, 6, 15, strips=True)
            emit_e(8, 15)
            emit_pair(0, 6, 11, strips=False)
            emit_pair(0, 11, 15, strips=False, stop=True)

            # drain PSUM to a scalar
            nc.vector.tensor_reduce(colsb[:, 0:1], acc[:, 0:224],
                                    mybir.AxisListType.X, Alu.add)
            nc.sync.dma_start(out=out_sum[:, :], in_=colsb[:, :])
    _split_multiwaits(nc)
    return nc


_NC_CACHE = None
LAST_RESULTS = None  # BassKernelResults of the most recent run (for test.py)


def kernel(sr_tensor: np.ndarray, hr_tensor: np.ndarray) -> np.ndarray:
    from concourse.bass_utils import run_bass_kernel_spmd

    global _NC_CACHE, LAST_RESULTS
    if _NC_CACHE is None:
        _NC_CACHE = _build_bass()
    nc = _NC_CACHE

    sr = np.asarray(sr_tensor, dtype=np.float32).reshape(H, W)
    hr = np.asarray(hr_tensor, dtype=np.float32).reshape(H, W)

    in_maps = []
    for c in range(NCORES):
        c0 = c * WC
        # [2048, 256] -> [128 patch-rows, 16 rows, 256 cols] -> interleave
        s16 = sr[:, c0:c0 + WC].reshape(128, FREE).astype(np.float16)
        h16 = hr[:, c0:c0 + WC].reshape(128, FREE).astype(np.float16)
        xi = np.empty((128, FREE, 2), dtype=np.float16)
        xi[:, :, 0] = s16
        xi[:, :, 1] = h16
        xi = xi.reshape(128, 2 * FREE)
        # block-major: A half (cols 0:4096) then B half, each [128, 4096]
        xs = np.concatenate([xi[:, 0:4096], xi[:, 4096:8192]], axis=0)
        in_maps.append({"x_in": np.ascontiguousarray(xs)})

    res = run_bass_kernel_spmd(nc, in_maps, list(range(NCORES)))
    LAST_RESULTS = res

    total = 0.0
    for r in res.results:
        total += float(np.asarray(r["out_sum"], dtype=np.float64)[0, 0])
    return np.float32(total / N_TERMS)


# revision 25
# speedup vs baseline: 1.1855x; 1.0866x over previous
"""Trainium2 Bass kernel for nn_DistanceLoss (patch neighbor-distance loss).

Reference semantics (k=16, H=W=2048, LOSS_WEIGHT=1):
  split each image into non-overlapping 16x16 patches; for interior pixels
  (local i,j in 1..14) and the 8-neighbor offset list [E,NW,NE,N,E,SW,SE,S]
  (E twice, W missing), accumulate || |sr_c-sr_n| - |hr_c-hr_n| || and take
  the global mean over L*14*14*8 terms.

Core trick: the per-term value t = ||u|-|v|| (u = sr_c-sr_n, v = hr_c-hr_n)
is three chained ABSOLUTE_DIFF ALU ops. The host stages sr/hr INTERLEAVED
(x[2f]=sr, x[2f+1]=hr) so that in the DVE's fp16 2x packed mode one
instruction sees all four operands per cycle (SRC_0=sr_x, SRC_0_HI=hr_x,
SRC_1=sr_{x+o}, SRC_1_HI=hr_{x+o}) and emits t duplicated to both write
lanes. This toolchain's walrus predates the CUSTOM_DVE_ANT opcodes, so the
custom 3-stage uop program is installed by HIJACKING the stock
TENSOR_TENSOR_ARITH_OP (0x41) row of the per-NEFF DVE table (the stock
sequencer handler already enables the two-source perf mode, which makes the
engine pick the 2x_1P uop slot for fp16 contiguous operands). Every
nc.vector.tensor_tensor in this kernel therefore computes the fused
pair-absdiff, one t per cycle per lane -- there is no S/D construction, no
shifted-copy DMA, no separate abs or min pass, and the Scalar engine is
freed up to issue half the input DMAs.

Opposite offsets +o/-o share one t array (sum over shifted windows), so the
pairs {N,S}, {NW,SE}, {NE,SW} cost one pass each and E (listed twice) has
weight 2. The interior-window sums run on PE as ones/twos-weighted
[128,1]^T @ t-row matmuls accumulating into PSUM [1,224]; rhs APs read the
duplicated t tiles with an inner stride of 2 so each t counts once. The
last pair (E) writes two tiles so PE can start its tail matmuls early.

Sharding: 256 image columns per core (16 patch-cols x 128 patch-rows),
free index f = i*256 + c; every neighbor offset is the constant
interleaved shift 2*(di*256+dj), always 4-byte aligned.
"""

import numpy as np

H = W = 2048
K = 16
NCORES = 8
WC = W // NCORES          # 256 columns per core
FREE = K * WC             # 4096 f-positions per partition
WIN = 15 * WC             # 3840: compute window covers i = 0..14
XPAD = 8208               # x tile width (2*FREE + junk tail for o=257 reads)
N_TERMS = (H // K) * (W // K) * (K - 2) * (K - 2) * 8
SPLIT_A = 1536            # A/B pass boundary (multiple of 256: row-aligned)

TT_ARITH_OPCODE = 0x41    # stock TENSOR_TENSOR_ARITH_OP row we repoint
PAIRMIN_NAME = "PAIRMIN_TT_ANT"


def _split_multiwaits(nc):
    """The walrus build here accepts at most one sync wait (and one update)
    per instruction: hoist extra waits onto same-engine NoOps inserted
    before the instruction, and extra updates onto NoOps after it."""
    from concourse import mybir

    k = 0
    for f in nc.m.functions:
        for bb in f.blocks:
            out, changed = [], False
            for i in bb.instructions:
                si = i.sync_info
                waits = list(si.on_wait) if si else []
                ups = list(si.on_update) if si else []
                trimmed = False
                if len(waits) > 1:
                    for w in waits[:-1]:
                        n = mybir.InstNoOp(name=f"{i.name}-sw{k}", ins=[],
                                           outs=[])
                        k += 1
                        n.engine = i.engine
                        n.sync_info = mybir.SyncInfo(on_wait=[w], on_update=[])
                        out.append(n)
                    waits, changed, trimmed = waits[-1:], True, True
                out.append(i)
                if len(ups) > 1:
                    i.sync_info = mybir.SyncInfo(on_wait=waits,
                                                 on_update=ups[:1])
                    for u in ups[1:]:
                        n = mybir.InstNoOp(name=f"{i.name}-su{k}", ins=[],
                                           outs=[])
                        k += 1
                        n.engine = i.engine
                        n.sync_info = mybir.SyncInfo(on_wait=[], on_update=[u])
                        out.append(n)
                    changed = True
                elif trimmed:
                    i.sync_info = mybir.SyncInfo(on_wait=waits, on_update=ups)
            if changed:
                bb.instructions = out
    return k


def _pairmin_ref(in0, in1, s0, s1, imm2):
    """numpy semantics of the hijacked op (sim/IR reference)."""
    a, b = in0[..., 0::2].astype(np.float32), in0[..., 1::2].astype(np.float32)
    c, d = in1[..., 0::2].astype(np.float32), in1[..., 1::2].astype(np.float32)
    t = np.abs(np.abs(a - c) - np.abs(b - d))
    return np.repeat(t, 2, axis=-1)


def _register_pairmin():
    """Install PAIRMIN into dve_ops.OPS with a hand-built 2x_1P uop program
    keyed to the stock TENSOR_TENSOR opcode row."""
    from concourse.dve_spec import Spec, Src0, Src1, Bin, lower
    from concourse.dve_uop import (
        UopConfig, DveOpSpec, InpSel, OutPath, OutSel,
        AluInp, AluOp, DelayInp, Trigger, ENABLE,
    )
    from concourse.dve_ops import DveOp, OPS, CUSTOM_DVE_SPECS, _COMPILE_CACHE

    if any(op.name == PAIRMIN_NAME for op in OPS):
        return

    u = UopConfig()
    u.inp[0], u.inp_enable[0] = InpSel.SRC_0, ENABLE       # sr_x
    u.inp[1], u.inp_enable[1] = InpSel.SRC_1, ENABLE       # sr_{x+o}
    u.inp[2], u.inp_enable[2] = InpSel.SRC_0_HI, ENABLE    # hr_x
    u.inp[3], u.inp_enable[3] = InpSel.SRC_1_HI, ENABLE    # hr_{x+o}
    dp = u.datapath_config
    dp[0].enable_alu(AluOp.ABSOLUTE_DIFF, AluInp.PREV_ALU_OUT,
                     AluInp.PREV_DELAY_0)
    dp[0].pass_through_delay(1, 2)
    dp[1].enable_alu(AluOp.ABSOLUTE_DIFF, AluInp.PREV_DELAY_1,
                     AluInp.PREV_DELAY_2)
    dp[1].enable_delay_from_src(DelayInp.PREV_ALU_OUT, 0)
    dp[2].enable_alu(AluOp.ABSOLUTE_DIFF, AluInp.PREV_ALU_OUT,
                     AluInp.PREV_DELAY_0)
    for k in range(3, 8):
        dp[k].pass_through_alu()
    u.out[OutPath.WR0_LO], u.out_enable[OutPath.WR0_LO] = OutSel.ALU_OUT, ENABLE
    u.out[OutPath.WR0_HI], u.out_enable[OutPath.WR0_HI] = OutSel.ALU_OUT, ENABLE
    u.require_inp0 = 1
    u.require_inp1 = 1
    u.trigger = (Trigger.SRC_TENSOR_DONE, Trigger.NONE, Trigger.NONE)

    op = DveOp(PAIRMIN_NAME,
               Spec(body=Bin(AluOp.ABSOLUTE_DIFF, Src0, Src1),
                    reference=_pairmin_ref),
               subdim=False, uops_sha={})
    OPS.append(op)
    CUSTOM_DVE_SPECS[PAIRMIN_NAME] = op.spec
    reg = lower(op.spec, ver="v3")
    assert len(reg) == 1
    _COMPILE_CACHE[(PAIRMIN_NAME, "v3")] = DveOpSpec(
        name=PAIRMIN_NAME, opcode=TT_ARITH_OPCODE, uops=reg,
        uops_2x=[u], perf_max=1, rd1_en=True)


def _build_bass():
    from concourse import bass, mybir, tile

    _register_pairmin()

    nc = bass.Bass()
    # Three DRAM-contiguous input blocks: the A half (cols 0:4096, sync
    # queue, split 2816/1280 so the first sub-passes can start early) and
    # the B half (scalar queue: ~3us doorbell latency, but B isn't needed
    # until ~8us after A). Fat sequential-HBM descriptors throughout.
    xa1 = nc.declare_dram_parameter("xa1", [128, 2816],
                                    mybir.dt.float16, isOutput=False)
    xa2 = nc.declare_dram_parameter("xa2", [128, 1280],
                                    mybir.dt.float16, isOutput=False)
    xb = nc.declare_dram_parameter("xb", [128, 4096],
                                   mybir.dt.float16, isOutput=False)
    out_sum = nc.declare_dram_parameter("out_sum", [1, 8],
                                        mybir.dt.float32, isOutput=True)
    nc.m.ant_custom_dve_ops = sorted({*nc.m.ant_custom_dve_ops, PAIRMIN_NAME})

    fp16 = mybir.dt.float16
    f32 = mybir.dt.float32
    Alu = mybir.AluOpType

    with tile.TileContext(nc) as tc:
        with tc.tile_pool(name="io", bufs=1) as io_pool, \
             tc.tile_pool(name="tpool", bufs=1) as t_pool, \
             tc.tile_pool(name="psum", bufs=1, space="PSUM") as psum_pool:
            x = io_pool.tile([128, XPAD], fp16, tag="x")
            w1 = io_pool.tile([128, 1], fp16, tag="w1")
            w2 = io_pool.tile([128, 1], fp16, tag="w2")
            acc = psum_pool.tile([1, 256], f32, tag="acc")
            colsb = io_pool.tile([1, 8], f32, tag="colsb")

            nc.vector.memset(w1[:, :], 1.0)
            nc.vector.memset(w2[:, :], 2.0)

            # A blocks on sync, B block on scalar (transfers run
            # concurrently on separate DGE queues).
            nc.sync.dma_start(out=x[:, 0:2816], in_=xa1[:, :])
            nc.sync.dma_start(out=x[:, 2816:4096], in_=xa2[:, :])
            nc.scalar.dma_start(out=x[:, 4096:8192], in_=xb[:, :])

            wts15 = [(1.0 if (i == 0 or i == 14) else 2.0)
                     for i in range(15)]

            # Per pair: neighbor offset o, trimmed j-window [jlo, jlo+jw)
            # covering exactly the columns the PE plan reads, mid slice
            # (a, b) in trimmed coords, strips as (elem_idx, row_lo, row_hi).
            P0 = dict(o=256, jlo=1, jw=14, mid=(0, 14), strips=[])
            P1 = dict(o=255, jlo=1, jw=15, mid=(1, 14),
                      strips=[(0, 1, 15), (28, 0, 14)])
            P2 = dict(o=257, jlo=0, jw=15, mid=(1, 14),
                      strips=[(28, 1, 15), (0, 0, 14)])
            PE_ = dict(o=1, jlo=1, jw=14)

            first_mm = [True]

            def mm(rhs, wts, stop=False):
                width = int(np.prod(rhs.shape[1:]))
                nc.tensor.matmul(acc[:, 0:width], wts[:, :], rhs,
                                 start=first_mm[0], stop=stop)
                first_mm[0] = False

            def xg(off):
                """x viewed as 32-element (i,q)-groups starting at `off`."""
                return x[:, off:off + 32 * 240].rearrange(
                    "p (g e) -> p g e", e=32)

            # fused pair-absdiff pass over groups [g0, g1): one hijacked
            # tensor_tensor on the interleaved tile, reading only the
            # trimmed j-window of each group. dst holds (t,t) pairs.
            def pair_pass(t_tile, dst_g0, g0, g1, P):
                o, jlo, jw = P["o"], P["jlo"], P["jw"]
                s0 = xg(2 * jlo)[:, g0:g1, 0:2 * jw]
                s1 = xg(2 * (o + jlo))[:, g0:g1, 0:2 * jw]
                tv = t_tile.rearrange("p (g e) -> p g e", e=2 * jw)
                nc.vector.tensor_tensor(tv[:, g0 - dst_g0:g1 - dst_g0, :],
                                        s0, s1, Alu.add)

            t256 = t_pool.tile([128, 240 * 28], fp16, tag="t0")
            t255 = t_pool.tile([128, 240 * 30], fp16, tag="t1")
            t257 = t_pool.tile([128, 240 * 30], fp16, tag="t2")
            t_a = t_pool.tile([128, 112 * 28], fp16, tag="ta")  # E rows 1-7
            t_b = t_pool.tile([128, 112 * 28], fp16, tag="tb")  # E rows 8-14

            # A0 passes need cols < 2816 (g < 72); A1 passes the full A half
            pair_pass(t256, 0, 0, 72, P0)
            pair_pass(t255, 0, 0, 72, P1)
            pair_pass(t257, 0, 0, 72, P2)
            pair_pass(t256, 0, 72, 96, P0)
            pair_pass(t255, 0, 72, 96, P1)
            pair_pass(t257, 0, 72, 96, P2)
            pair_pass(t_a, 16, 16, 128, PE_)

            # B phase (needs the full input), ordered so heavy-PE passes
            # finish first and the final pass (p256 rows 11-14) leaves only
            # four matmuls trailing.
            pair_pass(t255, 0, 96, 240, P1)
            pair_pass(t257, 0, 96, 240, P2)
            pair_pass(t_b, 128, 128, 240, PE_)
            pair_pass(t256, 0, 96, 176, P0)     # rows 6-10
            pair_pass(t256, 0, 176, 240, P0)    # rows 11-14

            # PE reductions. Views: i rows x 16 patches x jw cols x 2 dups.
            def views(tile_, jw):
                v5 = tile_.rearrange("p (i q j d) -> p i q j d",
                                     q=16, j=jw, d=2)
                v4 = tile_.rearrange("p (i q jd) -> p i q jd",
                                     q=16, jd=2 * jw)
                return v5, v4

            TP = [(t256, P0), (t255, P1), (t257, P2)]

            def emit_pair(pi, rlo, rhi, strips, stop=False):
                t, P = TP[pi]
                v5, v4 = views(t, P["jw"])
                a, b = P["mid"]
                for i in range(rlo, rhi):
                    w = w1 if wts15[i] == 1.0 else w2
                    mm(v5[:, i, :, a:b, 0:1], w, stop=stop and i == rhi - 1)
                if strips:
                    for idx, slo, shi in P["strips"]:
                        mm(v4[:, slo:shi, :, idx:idx + 1], w1)

            va5, _ = views(t_a, 14)
            vb5, _ = views(t_b, 14)

            def emit_e(rlo, rhi):
                for i in range(rlo, rhi):
                    v = va5[:, i - 1] if i < 8 else vb5[:, i - 8]
                    mm(v[:, :, :, 0:1], w2)

            # availability order matching the DVE pass schedule above
            emit_pair(0, 0, 4, strips=False)
            emit_pair(1, 0, 4, strips=False)
            emit_pair(2, 0, 4, strips=False)
            emit_pair(0, 4, 6, strips=False)
            emit_pair(1, 4, 6, strips=False)
            emit_pair(2, 4, 6, strips=False)
            emit_e(1, 8)
            emit_pair(1, 6, 15, strips=True)
            emit_pair(2, 6, 15, strips=True)
            emit_e(8, 15)
            emit_pair(0, 6, 11, strips=False)
            emit_pair(0, 11, 15, strips=False, stop=True)

            # drain PSUM to a scalar
            nc.vector.tensor_reduce(colsb[:, 0:1], acc[:, 0:224],
                                    mybir.AxisListType.X, Alu.add)
            nc.sync.dma_start(out=out_sum[:, :], in_=colsb[:, :])
    _split_multiwaits(nc)
    return nc


_NC_CACHE = None
LAST_RESULTS = None  # BassKernelResults of the most recent run (for test.py)


def kernel(sr_tensor: np.ndarray, hr_tensor: np.ndarray) -> np.ndarray:
    from concourse.bass_utils import run_bass_kernel_spmd

    global _NC_CACHE, LAST_RESULTS
    if _NC_CACHE is None:
        _NC_CACHE = _build_bass()
    nc = _NC_CACHE

    sr = np.asarray(sr_tensor, dtype=np.float32).reshape(H, W)
    hr = np.asarray(hr_tensor, dtype=np.float32).reshape(H, W)

    in_maps = []
    for c in range(NCORES):
        c0 = c * WC
        # [2048, 256] -> [128 patch-rows, 16 rows, 256 cols] -> interleave
        s16 = sr[:, c0:c0 + WC].reshape(128, FREE).astype(np.float16)
        h16 = hr[:, c0:c0 + WC].reshape(128, FREE).astype(np.float16)
        xi = np.empty((128, FREE, 2), dtype=np.float16)
        xi[:, :, 0] = s16
        xi[:, :, 1] = h16
        xi = xi.reshape(128, 2 * FREE)
        in_maps.append({
            "xa1": np.ascontiguousarray(xi[:, 0:2816]),
            "xa2": np.ascontiguousarray(xi[:, 2816:4096]),
            "xb": np.ascontiguousarray(xi[:, 4096:8192]),
        })

    res = run_bass_kernel_spmd(nc, in_maps, list(range(NCORES)))
    LAST_RESULTS = res

    total = 0.0
    for r in res.results:
        total += float(np.asarray(r["out_sum"], dtype=np.float64)[0, 0])
    return np.float32(total / N_TERMS)
